# revision 23
# baseline (speedup 1.0000x reference)
"""Trainium2 Bass kernel for nn_EntropyCalculator (per-row histogram entropy).

x: [262144, 64] int32, values in [0, 40). Output: [262144, 1] float32 per-row
entropy of the value histogram: -sum_v p_v*log(p_v + 1e-8), p = c/(64+1e-8).

Multi-engine strategy (per core, pure data parallel over 8 cores):
  The 40-bin histogram is computed as 14 "limb" channels; limb g packs the
  counts of values {3g, 3g+1, 3g+2} into one fp32 number c0 + 128*c1 + c2/128
  (exact: 21 bits < fp32's 24-bit mantissa). The per-element encode is the
  window parabola P(t) = 1 + A*t + B*t^2 (t = x - 3g), which equals the digit
  weight at t = 0,1,2 and is negative at every other integer in range.

  Channels are split across engines:
   * N_V channels run on VectorE as one fused custom DVE op per channel
     (relu-parabola + prefix scan; per-row sums via boundary differences).
   * N_OFF channels run on the transposed path: TensorE transposes x-blocks
     into PSUM; ScalarE computes u = Square(sqrt|B|*(x - mu_g)) (the free
     affine input of ACT); a one-op tensor_scalar finisher (GP or VE) forms
     z = min(u - h, 0) in {0, -1, -128, -1/128} (bf16-exact); TensorE then
     row-sums z by loading it as the stationary operand and multiplying a
     [-1] moving vector, landing packed sums in PSUM with rows on partitions
     (so no partition-crossing repack is needed).
  Decode (digit extraction via exact rint, ACT-Ln, fused multiply-scans) is
  shared, operating on the merged [P, rows, 14] layout.
"""

import numpy as np

VOCAB = 40
L = 64
B = 262144
NCORES = 8
ROWS_PC = B // NCORES          # 32768 rows per core
P = 128                        # SBUF partitions
RPP = ROWS_PC // P             # 256 rows per partition
RC = 32                        # rows per partition per chunk
NCHUNK = RPP // RC             # 8 chunks
SCANROWS = 16                  # rows per scan instruction (fp32 exactness cap)
NSUB = RC // SCANROWS          # 2 scan sub-chunks per chunk
NLIMB = 14
EPS = 1e-8
S_PRIME = 64.0 + EPS

# --- engine split config (tunable) ---
N_V = 5                        # limbs 0..N_V-1 on VectorE fused scans
N_OFF = NLIMB - N_V            # limbs N_V.. on the transposed SE/TE path
N_F_VE = 14                    # of the offloaded, how many finishers on VE
CONV_ENGINE = "vector"         # int32->bf16 conversion engine: gpsimd|vector|scalar
STAGE1_PSUM_ENGINE = "gpsimd"  # psum-side digit stage helpers engine for tt ops

# parabola through (1, 128, 1/128) at t=0,1,2; negative at all other ints
A_C = 254.49609375
B_C = -127.49609375
ABS_B = -B_C                               # 127.49609375
V_C = A_C / (2.0 * ABS_B)                  # vertex offset ~0.99806
H_C = 1.0 + A_C * A_C / (4.0 * ABS_B)     # peak value ~128.0078
SQRT_B = float(np.sqrt(ABS_B))
MAGIC = 8388608.0              # 2^23: rint via (x + 2^23) - 2^23

_RUNNER = None


def _register_ops():
    import concourse.dve_ops as dve_ops
    from concourse.dve_spec import (
        Spec, Src0, Src1, C0, C1, C2, One, scan, AluOp, lower, _has_src1, sq,
        relu,
    )
    from concourse.dve_uop import DveOpSpec

    def reg(name, spec, subdim=False):
        for op in dve_ops.OPS:
            if op.name == name:
                return op
        row = dve_ops._CUSTOM_DVE_ROW_BASE + len(dve_ops.OPS)
        assert row < 0x20, "out of custom-DVE opcode rows"
        shas = {}
        for ver in ("v3", "v4"):
            s = DveOpSpec(name=name, opcode=row, uops=lower(spec, ver=ver),
                          rd1_en=_has_src1(spec))
            shas[ver] = s.sha(ver)
        op = dve_ops.DveOp(name, spec, subdim=subdim, uops_sha=shas)
        dve_ops.OPS.append(op)
        dve_ops.CUSTOM_DVE_SPECS[name] = spec
        dve_ops._SUB_OPCODE_FOR_NAME[name] = row
        return op

    _t = Src0 - C0

    def _ref_limb(in0, in1, s0, s1, imm2):
        t = in0.astype(np.float64) - s0
        z = np.maximum(1.0 + t * s1 + t * t * imm2, 0.0)
        return np.cumsum(z.reshape(z.shape[0], -1), axis=1).astype(np.float32)

    limb = reg("ENT_LIMB_SCAN", Spec(
        body=scan(AluOp.ADD, relu(One + _t * C1 + sq(_t) * C2)),
        reference=_ref_limb))

    def _ref_rint(in0, in1, s0, s1, imm2):
        y = (in0.astype(np.float32) * np.float32(s0)) - np.float32(s1)
        return ((y + np.float32(imm2)) - np.float32(imm2)).astype(np.float32)

    rint = reg("ENT_RINT_AFFINE", Spec(
        body=(Src0 * C0 - C1 + C2) - C2,
        reference=_ref_rint))

    def _ref_dot(in0, in1, s0, s1, imm2):
        z = in0.astype(np.float64) * in1.astype(np.float64)
        return np.cumsum(z.reshape(z.shape[0], -1), axis=1).astype(np.float32)

    dot = reg("ENT_DOT_SCAN", Spec(
        body=scan(AluOp.ADD, Src0 * Src1),
        reference=_ref_dot))

    return limb, rint, dot


def _build_nc(repeat=1):
    from contextlib import ExitStack, nullcontext
    import concourse.bacc as bacc
    import concourse.mybir as mybir
    from concourse.tile import TileContext
    from concourse import masks

    LIMB, RINT, DOT = _register_ops()
    dt = mybir.dt
    Alu = mybir.AluOpType
    Act = mybir.ActivationFunctionType

    nc = bacc.Bacc()
    x = nc.dram_tensor("x", [ROWS_PC, L], dt.bfloat16, kind="ExternalInput")
    y = nc.dram_tensor("y", [ROWS_PC, 1], dt.float32, kind="ExternalOutput")

    # partition p owns rows [p*RPP, (p+1)*RPP); chunk c covers rows c*RC..+RC
    xv = x[:].rearrange("(p c r) l -> p c (r l)", p=P, c=NCHUNK)   # [P, NCHUNK, RC*L]
    yv = y[:].rearrange("(p c r) o -> p c (r o)", p=P, c=NCHUNK)   # [P, NCHUNK, RC]

    NA = RC * NLIMB            # 448 decode values per partition per chunk
    inv_sp = float(1.0 / S_PRIME)
    NJP = RC // 2              # 16 j-pairs per chunk

    eng = {"gpsimd": nc.gpsimd, "vector": nc.vector, "scalar": nc.scalar}

    with TileContext(nc) as tc:
        with ExitStack() as ctx:
            xpool = ctx.enter_context(tc.tile_pool(name="xp", bufs=3))
            ppool = ctx.enter_context(tc.tile_pool(name="pp", bufs=3))
            upool = ctx.enter_context(tc.tile_pool(name="up", bufs=4))
            zpool = ctx.enter_context(tc.tile_pool(name="zp", bufs=4))
            apool = ctx.enter_context(tc.tile_pool(name="ap", bufs=2))
            dpool = ctx.enter_context(tc.tile_pool(name="dp", bufs=2))
            epool = ctx.enter_context(tc.tile_pool(name="ep", bufs=2))
            tpsum = ctx.enter_context(tc.tile_pool(name="tp", bufs=2,
                                                   space="PSUM"))
            spsum = ctx.enter_context(tc.tile_pool(name="sp", bufs=2,
                                                   space="PSUM"))
            singles = ctx.enter_context(tc.tile_pool(name="sg", bufs=1))

            t_eps = singles.tile([P, 1], dt.float32)
            nc.vector.memset(t_eps[:], EPS)
            t_inv = singles.tile([P, 1], dt.float32)
            nc.vector.memset(t_inv[:], inv_sp)
            t_inv128 = singles.tile([P, 1], dt.float32)
            nc.vector.memset(t_inv128[:], float(128.0 / S_PRIME))

            ident = singles.tile([P, P], dt.bfloat16)
            masks.make_identity(nc, ident[:])

            t_sqb = singles.tile([P, 1], dt.float32)
            nc.vector.memset(t_sqb[:], float(SQRT_B))
            t_bias = []
            for gi in range(N_OFF):
                mu = 3.0 * (N_V + gi) + V_C
                tb = singles.tile([P, 1], dt.float32, tag=f"tb{gi}")
                nc.vector.memset(tb[:], float(-SQRT_B * mu))
                t_bias.append(tb)

            # moving operand for row sums: col e has -1 on partitions
            # [64e, 64e+64) and 0 elsewhere
            onesm = singles.tile([P, 2], dt.bfloat16)
            nc.vector.memset(onesm[:], 0.0)
            nc.vector.memset(onesm[0:64, 0:1], -1.0)
            nc.vector.memset(onesm[64:128, 1:2], -1.0)

            repctx = tc.For_i(0, repeat, 1) if repeat > 1 else nullcontext()
            with repctx:
              def stage_a(c):
                xt = xpool.tile([P, RC * L], dt.bfloat16, tag="x")
                nc.sync.dma_start(out=xt[:], in_=xv[:, c, :])

                # x_T[64e+l, jp*128+q] = x[row q*RPP + c*RC + 2jp+e, l]
                xT = tpsum.tile([P, NJP * P], dt.bfloat16, tag="xT")
                for jp in range(NJP):
                    nc.tensor.matmul(
                        out=xT[:, jp * P:(jp + 1) * P],
                        lhsT=xt[:, jp * 2 * L:(jp + 1) * 2 * L],
                        rhs=ident[:],
                        is_transpose=True)

                # VE scan path issued BEFORE the finishers so VectorE has
                # immediate work while ScalarE runs the squares.
                AbV = apool.tile([P, NSUB, SCANROWS, N_V], dt.float32,
                                 tag="AV")
                SL = SCANROWS * L
                for g in range(N_V):
                    pref = ppool.tile([P, NSUB, 1 + SL], dt.float32,
                                      tag="pref")
                    nc.gpsimd.memset(pref[:, :, 0:1], 0.0)
                    for s in range(NSUB):
                        nc.vector._custom_dve(
                            LIMB,
                            out=pref[:, s, 1:],
                            in0=xt[:, s * SL:(s + 1) * SL],
                            s0=float(3 * g), s1=A_C, imm2=B_C)
                    nc.gpsimd.tensor_tensor(
                        out=AbV[:, :, :, g],
                        in0=pref[:, :, L::L],
                        in1=pref[:, :, 0:SL:L],
                        op=Alu.subtract)

                # packed sums tile: [P=q, g, jp, e] (fp32)
                sE = spsum.tile([P, N_OFF, NJP, 2], dt.float32, tag="sE")
                for gi in range(N_OFF):
                    u = upool.tile([P, NJP * P], dt.float32, tag="u")
                    nc.scalar.activation(u[:], xT[:], Act.Square,
                                         bias=t_bias[gi][:],
                                         scale=t_sqb[:])
                    z = zpool.tile([P, NJP * P], dt.bfloat16, tag="z")
                    fin = nc.vector if gi < N_F_VE else nc.gpsimd
                    fin.tensor_scalar(out=z[:], in0=u[:],
                                      scalar1=float(H_C), scalar2=0.0,
                                      op0=Alu.subtract, op1=Alu.min)
                    for jp in range(NJP):
                        nc.tensor.matmul(
                            out=sE[:, gi, jp, :],
                            lhsT=z[:, jp * P:(jp + 1) * P],
                            rhs=onesm[:],
                            start=True, stop=True)
                return AbV, sE

              def stage_b(c, AbV, sE):
                # ---------- merged digit decode ----------
                # merged layout: [P, r(RC), g(NLIMB)]
                c0m = dpool.tile([P, RC, NLIMB], dt.float32, tag="c0")
                c1m = dpool.tile([P, RC, NLIMB], dt.float32, tag="c1")
                ddm = dpool.tile([P, RC, NLIMB], dt.float32, tag="dd")

                def stage1(src_flat, src_shaped, reshape, dst_c0, dst_c1,
                           dst_dd, n, tag, tt_eng):
                    ri = dpool.tile([P, n], dt.float32, tag=tag + "ri")
                    c1l = dpool.tile([P, n], dt.float32, tag=tag + "c1")
                    nc.vector._custom_dve(RINT, out=ri[:], in0=src_flat,
                                          s0=1.0, s1=0.25, imm2=MAGIC)
                    tt_eng.tensor_tensor(out=dst_dd, in0=src_shaped,
                                         in1=reshape(ri[:]),
                                         op=Alu.subtract)
                    nc.vector._custom_dve(RINT, out=c1l[:], in0=ri[:],
                                          s0=0.0078125, s1=0.25, imm2=MAGIC)
                    nc.vector.scalar_tensor_tensor(
                        out=dst_c0, in0=reshape(c1l[:]), scalar=-128.0,
                        in1=reshape(ri[:]), op0=Alu.mult, op1=Alu.add)
                    nc.vector.tensor_copy(dst_c1, reshape(c1l[:]))

                # VE side: AbV [P, (s r), g] is r-major ((s r) = j)
                stage1(AbV[:].rearrange("p s r g -> p (s r g)"),
                       AbV[:].rearrange("p s r g -> p (s r) g"),
                       lambda a: a.rearrange("p (r g) -> p r g", g=N_V),
                       c0m[:, :, 0:N_V], c1m[:, :, 0:N_V], ddm[:, :, 0:N_V],
                       RC * N_V, "v", nc.gpsimd)

                # psum side: sE [P, g, jp, e]; dst row r = 2*jp+e, limb N_V+g
                sEs = dpool.tile([P, N_OFF, NJP, 2], dt.float32, tag="sEs")
                nc.scalar.copy(sEs[:], sE[:])
                resh = lambda a: a.rearrange("p (g jp e) -> p g jp e",
                                             jp=NJP, e=2)
                stage1(sEs[:].rearrange("p g jp e -> p (g jp e)"),
                       sEs[:],
                       resh,
                       c0m[:, :, N_V:].rearrange("p (jp e) g -> p g jp e",
                                                 jp=NJP),
                       c1m[:, :, N_V:].rearrange("p (jp e) g -> p g jp e",
                                                 jp=NJP),
                       ddm[:, :, N_V:].rearrange("p (jp e) g -> p g jp e",
                                                 jp=NJP),
                       RC * N_OFF, "o", eng[STAGE1_PSUM_ENGINE])

                # u_i = log(c_i/S' + eps); dd lane folds 128 into scale/combine
                u0 = dpool.tile([P, NA], dt.float32, tag="u0")
                u1 = dpool.tile([P, NA], dt.float32, tag="u1")
                u2 = dpool.tile([P, NA], dt.float32, tag="u2")
                c0f = c0m[:].rearrange("p r g -> p (r g)")
                c1f = c1m[:].rearrange("p r g -> p (r g)")
                ddf = ddm[:].rearrange("p r g -> p (r g)")
                nc.scalar.activation(u0[:], c0f, Act.Ln,
                                     bias=t_eps[:], scale=t_inv[:])
                nc.scalar.activation(u1[:], c1f, Act.Ln,
                                     bias=t_eps[:], scale=t_inv[:])
                nc.scalar.activation(u2[:], ddf, Act.Ln,
                                     bias=t_eps[:], scale=t_inv128[:])

                d0 = dpool.tile([P, 1 + NA], dt.float32, tag="d0")
                d1 = dpool.tile([P, 1 + NA], dt.float32, tag="d1")
                d2 = dpool.tile([P, 1 + NA], dt.float32, tag="d2")
                for (dx, cf, uf) in ((d0, c0f, u0), (d1, c1f, u1),
                                     (d2, ddf, u2)):
                    nc.gpsimd.memset(dx[:, 0:1], 0.0)
                    nc.vector._custom_dve(DOT, out=dx[:, 1:], in0=cf,
                                          in1=uf[:])

                # per-row sums from scan ends: S[r] = d[r,13] - d[r-1,13]
                e0 = epool.tile([P, RC], dt.float32, tag="e0")
                e1 = epool.tile([P, RC], dt.float32, tag="e1")
                e2 = epool.tile([P, RC], dt.float32, tag="e2")
                for (dx, ex) in ((d0, e0), (d1, e1), (d2, e2)):
                    nc.gpsimd.tensor_tensor(
                        out=ex[:], in0=dx[:, NLIMB::NLIMB],
                        in1=dx[:, 0:NA:NLIMB], op=Alu.subtract)

                # entropy = -(E0 + E1 + 128*E2)/S'
                acc = epool.tile([P, RC], dt.float32, tag="acc")
                nc.vector.scalar_tensor_tensor(
                    out=acc[:], in0=e2[:], scalar=128.0, in1=e1[:],
                    op0=Alu.mult, op1=Alu.add)
                nc.gpsimd.tensor_tensor(out=acc[:], in0=acc[:], in1=e0[:],
                                        op=Alu.add)
                eout = epool.tile([P, RC], dt.float32, tag="eout")
                nc.vector.tensor_scalar_mul(eout[:], acc[:],
                                            float(-1.0 / S_PRIME))
                nc.sync.dma_start(out=yv[:, c, :], in_=eout[:])

              for c in range(NCHUNK):
                ab, se = stage_a(c)
                stage_b(c, ab, se)

    nc.finalize()
    return nc


def _build_runner(repeat=1):
    """Cached jitted 8-core runner (modeled on bass2jax.run_bass_via_pjrt,
    but reusing one jitted executable across calls)."""
    import jax
    from jax.sharding import Mesh, PartitionSpec
    from jax.experimental.shard_map import shard_map
    import concourse.bass2jax as b2j

    nc = _build_nc(repeat=repeat)
    b2j.install_neuronx_cc_hook()

    import concourse.mybir as mybir
    partition_name = (nc.partition_id_tensor.name
                      if nc.partition_id_tensor else None)
    in_names, out_names, out_avals, zero_outs = [], [], [], []
    for alloc in nc.m.functions[0].allocations:
        if not isinstance(alloc, mybir.MemoryLocationSet):
            continue
        name = alloc.memorylocations[0].name
        if alloc.kind == "ExternalInput":
            if name != partition_name:
                in_names.append(name)
        elif alloc.kind == "ExternalOutput":
            shape = tuple(alloc.tensor_shape)
            dtype = mybir.dt.np(alloc.dtype)
            out_names.append(name)
            out_avals.append(jax.core.ShapedArray(shape, dtype))
            zero_outs.append(np.zeros(shape, dtype))
    n_params = len(in_names)
    n_outs = len(out_avals)
    all_in_names = in_names + out_names
    if partition_name is not None:
        all_in_names = all_in_names + [partition_name]

    def _body(*args):
        operands = list(args)
        if partition_name is not None:
            operands.append(b2j.partition_id_tensor())
        outs = b2j._bass_exec_p.bind(
            *operands,
            out_avals=tuple(out_avals),
            in_names=tuple(all_in_names),
            out_names=tuple(out_names),
            lowering_input_output_aliases=(),
            sim_require_finite=True,
            sim_require_nnan=True,
            nc=nc,
        )
        return tuple(outs)

    devices = jax.devices()[:NCORES]
    mesh = Mesh(np.asarray(devices), ("core",))
    sharded = jax.jit(
        shard_map(_body, mesh=mesh,
                  in_specs=(PartitionSpec("core"),) * (n_params + n_outs),
                  out_specs=(PartitionSpec("core"),) * n_outs,
                  check_rep=False),
        donate_argnums=tuple(range(n_params, n_params + n_outs)),
        keep_unused=True,
    )

    def run(x_full: np.ndarray) -> np.ndarray:
        zeros = [np.zeros((NCORES * z.shape[0], *z.shape[1:]), z.dtype)
                 for z in zero_outs]
        out = sharded(x_full, *zeros)
        return np.asarray(out[0])

    run.sharded = sharded
    run.zero_outs = zero_outs
    run.mesh = mesh
    return run


def kernel(x: np.ndarray) -> np.ndarray:
    global _RUNNER
    import ml_dtypes
    x = np.asarray(x)
    assert x.shape == (B, L), x.shape
    # values are in [0, 40): exactly representable in bf16
    if x.dtype != ml_dtypes.bfloat16:
        x = x.astype(np.float32).astype(ml_dtypes.bfloat16)
    if _RUNNER is None:
        _RUNNER = _build_runner()
    try:
        out = _RUNNER(x)
    except Exception:
        # transient device hiccups (NRT exec-unit resets) have been observed
        # once on this fabric; one retry after a short pause recovers.
        import time
        time.sleep(20.0)
        out = _RUNNER(x)
    return out.reshape(B, 1).astype(np.float32)


if __name__ == "__main__":
    rng = np.random.default_rng(0)
    xa = rng.integers(0, VOCAB, size=(B, L)).astype(np.int32)
    out = kernel(x=xa)
    cnt = np.zeros((B, VOCAB), np.float64)
    for v in range(VOCAB):
        cnt[:, v] = (xa == v).sum(1)
    p = cnt / S_PRIME
    ref = -(p * np.log(p + EPS)).sum(1, keepdims=True)
    err = np.abs(out - ref).max()
    rel = err / np.abs(ref).max()
    print("selfcheck max abs err:", err, "rel:", rel)


# revision 25
# speedup vs baseline: 1.0026x; 1.0026x over previous
"""Trainium2 Bass kernel for nn_EntropyCalculator (per-row histogram entropy).

x: [262144, 64] int32, values in [0, 40). Output: [262144, 1] float32 per-row
entropy of the value histogram: -sum_v p_v*log(p_v + 1e-8), p = c/(64+1e-8).

Multi-engine strategy (per core, pure data parallel over 8 cores):
  The 40-bin histogram is computed as 14 "limb" channels; limb g packs the
  counts of values {3g, 3g+1, 3g+2} into one fp32 number c0 + 128*c1 + c2/128
  (exact: 21 bits < fp32's 24-bit mantissa). The per-element encode is the
  window parabola P(t) = 1 + A*t + B*t^2 (t = x - 3g), which equals the digit
  weight at t = 0,1,2 and is negative at every other integer in range.

  Channels are split across engines:
   * N_V channels run on VectorE as one fused custom DVE op per channel
     (relu-parabola + prefix scan; per-row sums via boundary differences).
   * N_OFF channels run on the transposed path: TensorE transposes x-blocks
     into PSUM; ScalarE computes u = Square(sqrt|B|*(x - mu_g)) (the free
     affine input of ACT); a one-op tensor_scalar finisher (GP or VE) forms
     z = min(u - h, 0) in {0, -1, -128, -1/128} (bf16-exact); TensorE then
     row-sums z by loading it as the stationary operand and multiplying a
     [-1] moving vector, landing packed sums in PSUM with rows on partitions
     (so no partition-crossing repack is needed).
  Decode (digit extraction via exact rint, ACT-Ln, fused multiply-scans) is
  shared, operating on the merged [P, rows, 14] layout.
"""

import numpy as np

VOCAB = 40
L = 64
B = 262144
NCORES = 8
ROWS_PC = B // NCORES          # 32768 rows per core
P = 128                        # SBUF partitions
RPP = ROWS_PC // P             # 256 rows per partition
RC = 32                        # rows per partition per chunk
NCHUNK = RPP // RC             # 8 chunks
SCANROWS = 16                  # rows per scan instruction (fp32 exactness cap)
NSUB = RC // SCANROWS          # 2 scan sub-chunks per chunk
NLIMB = 14
EPS = 1e-8
S_PRIME = 64.0 + EPS

# --- engine split config (tunable) ---
N_V = 5                        # limbs 0..N_V-1 on VectorE fused scans
N_OFF = NLIMB - N_V            # limbs N_V.. on the transposed SE/TE path
N_F_VE = 14                    # of the offloaded, how many finishers on VE
CONV_ENGINE = "vector"         # int32->bf16 conversion engine: gpsimd|vector|scalar
STAGE1_PSUM_ENGINE = "vector"  # psum-side digit stage helpers engine for tt ops

# parabola through (1, 128, 1/128) at t=0,1,2; negative at all other ints
A_C = 254.49609375
B_C = -127.49609375
ABS_B = -B_C                               # 127.49609375
V_C = A_C / (2.0 * ABS_B)                  # vertex offset ~0.99806
H_C = 1.0 + A_C * A_C / (4.0 * ABS_B)     # peak value ~128.0078
SQRT_B = float(np.sqrt(ABS_B))
MAGIC = 8388608.0              # 2^23: rint via (x + 2^23) - 2^23

_RUNNER = None


def _register_ops():
    import concourse.dve_ops as dve_ops
    from concourse.dve_spec import (
        Spec, Src0, Src1, C0, C1, C2, One, scan, AluOp, lower, _has_src1, sq,
        relu,
    )
    from concourse.dve_uop import DveOpSpec

    def reg(name, spec, subdim=False):
        for op in dve_ops.OPS:
            if op.name == name:
                return op
        row = dve_ops._CUSTOM_DVE_ROW_BASE + len(dve_ops.OPS)
        assert row < 0x20, "out of custom-DVE opcode rows"
        shas = {}
        for ver in ("v3", "v4"):
            s = DveOpSpec(name=name, opcode=row, uops=lower(spec, ver=ver),
                          rd1_en=_has_src1(spec))
            shas[ver] = s.sha(ver)
        op = dve_ops.DveOp(name, spec, subdim=subdim, uops_sha=shas)
        dve_ops.OPS.append(op)
        dve_ops.CUSTOM_DVE_SPECS[name] = spec
        dve_ops._SUB_OPCODE_FOR_NAME[name] = row
        return op

    _t = Src0 - C0

    def _ref_limb(in0, in1, s0, s1, imm2):
        t = in0.astype(np.float64) - s0
        z = np.maximum(1.0 + t * s1 + t * t * imm2, 0.0)
        return np.cumsum(z.reshape(z.shape[0], -1), axis=1).astype(np.float32)

    limb = reg("ENT_LIMB_SCAN", Spec(
        body=scan(AluOp.ADD, relu(One + _t * C1 + sq(_t) * C2)),
        reference=_ref_limb))

    def _ref_rint(in0, in1, s0, s1, imm2):
        y = (in0.astype(np.float32) * np.float32(s0)) - np.float32(s1)
        return ((y + np.float32(imm2)) - np.float32(imm2)).astype(np.float32)

    rint = reg("ENT_RINT_AFFINE", Spec(
        body=(Src0 * C0 - C1 + C2) - C2,
        reference=_ref_rint))

    def _ref_dot(in0, in1, s0, s1, imm2):
        z = in0.astype(np.float64) * in1.astype(np.float64)
        return np.cumsum(z.reshape(z.shape[0], -1), axis=1).astype(np.float32)

    dot = reg("ENT_DOT_SCAN", Spec(
        body=scan(AluOp.ADD, Src0 * Src1),
        reference=_ref_dot))

    return limb, rint, dot


def _build_nc(repeat=1):
    from contextlib import ExitStack, nullcontext
    import concourse.bacc as bacc
    import concourse.mybir as mybir
    from concourse.tile import TileContext
    from concourse import masks

    LIMB, RINT, DOT = _register_ops()
    dt = mybir.dt
    Alu = mybir.AluOpType
    Act = mybir.ActivationFunctionType

    nc = bacc.Bacc()
    x = nc.dram_tensor("x", [ROWS_PC, L], dt.int32, kind="ExternalInput")
    y = nc.dram_tensor("y", [ROWS_PC, 1], dt.float32, kind="ExternalOutput")

    # partition p owns rows [p*RPP, (p+1)*RPP); chunk c covers rows c*RC..+RC
    xv = x[:].rearrange("(p c r) l -> p c (r l)", p=P, c=NCHUNK)   # [P, NCHUNK, RC*L]
    yv = y[:].rearrange("(p c r) o -> p c (r o)", p=P, c=NCHUNK)   # [P, NCHUNK, RC]

    NA = RC * NLIMB            # 448 decode values per partition per chunk
    inv_sp = float(1.0 / S_PRIME)
    NJP = RC // 2              # 16 j-pairs per chunk

    eng = {"gpsimd": nc.gpsimd, "vector": nc.vector, "scalar": nc.scalar}

    with TileContext(nc) as tc:
        with ExitStack() as ctx:
            xpool = ctx.enter_context(tc.tile_pool(name="xp", bufs=3))
            bpool = ctx.enter_context(tc.tile_pool(name="bp", bufs=2))
            ppool = ctx.enter_context(tc.tile_pool(name="pp", bufs=3))
            upool = ctx.enter_context(tc.tile_pool(name="up", bufs=4))
            zpool = ctx.enter_context(tc.tile_pool(name="zp", bufs=4))
            apool = ctx.enter_context(tc.tile_pool(name="ap", bufs=2))
            dpool = ctx.enter_context(tc.tile_pool(name="dp", bufs=2))
            epool = ctx.enter_context(tc.tile_pool(name="ep", bufs=2))
            tpsum = ctx.enter_context(tc.tile_pool(name="tp", bufs=2,
                                                   space="PSUM"))
            spsum = ctx.enter_context(tc.tile_pool(name="sp", bufs=2,
                                                   space="PSUM"))
            singles = ctx.enter_context(tc.tile_pool(name="sg", bufs=1))

            t_eps = singles.tile([P, 1], dt.float32)
            nc.vector.memset(t_eps[:], EPS)
            t_inv = singles.tile([P, 1], dt.float32)
            nc.vector.memset(t_inv[:], inv_sp)
            t_inv128 = singles.tile([P, 1], dt.float32)
            nc.vector.memset(t_inv128[:], float(128.0 / S_PRIME))

            ident = singles.tile([P, P], dt.bfloat16)
            masks.make_identity(nc, ident[:])

            t_sqb = singles.tile([P, 1], dt.float32)
            nc.vector.memset(t_sqb[:], float(SQRT_B))
            t_bias = []
            for gi in range(N_OFF):
                mu = 3.0 * (N_V + gi) + V_C
                tb = singles.tile([P, 1], dt.float32, tag=f"tb{gi}")
                nc.vector.memset(tb[:], float(-SQRT_B * mu))
                t_bias.append(tb)

            # moving operand for row sums: col e has -1 on partitions
            # [64e, 64e+64) and 0 elsewhere
            onesm = singles.tile([P, 2], dt.bfloat16)
            nc.vector.memset(onesm[:], 0.0)
            nc.vector.memset(onesm[0:64, 0:1], -1.0)
            nc.vector.memset(onesm[64:128, 1:2], -1.0)

            repctx = tc.For_i(0, repeat, 1) if repeat > 1 else nullcontext()
            with repctx:
              def stage_a(c):
                xt = xpool.tile([P, RC * L], dt.int32, tag="x")
                nc.sync.dma_start(out=xt[:], in_=xv[:, c, :])

                # conversion first: unblocks TE transposes
                xb = bpool.tile([P, RC * L], dt.bfloat16, tag="xb")
                if CONV_ENGINE == "scalar":
                    nc.scalar.copy(xb[:], xt[:])
                else:
                    eng[CONV_ENGINE].tensor_copy(xb[:], xt[:])

                # x_T[64e+l, jp*128+q] = x[row q*RPP + c*RC + 2jp+e, l]
                xT = tpsum.tile([P, NJP * P], dt.bfloat16, tag="xT")
                for jp in range(NJP):
                    nc.tensor.matmul(
                        out=xT[:, jp * P:(jp + 1) * P],
                        lhsT=xb[:, jp * 2 * L:(jp + 1) * 2 * L],
                        rhs=ident[:],
                        is_transpose=True)

                # VE scan path issued BEFORE the finishers so VectorE has
                # immediate work while ScalarE runs the squares.
                AbV = apool.tile([P, NSUB, SCANROWS, N_V], dt.float32,
                                 tag="AV")
                SL = SCANROWS * L
                for g in range(N_V):
                    pref = ppool.tile([P, NSUB, 1 + SL], dt.float32,
                                      tag="pref")
                    nc.gpsimd.memset(pref[:, :, 0:1], 0.0)
                    for s in range(NSUB):
                        nc.vector._custom_dve(
                            LIMB,
                            out=pref[:, s, 1:],
                            in0=xt[:, s * SL:(s + 1) * SL],
                            s0=float(3 * g), s1=A_C, imm2=B_C)
                    nc.vector.tensor_tensor(
                        out=AbV[:, :, :, g],
                        in0=pref[:, :, L::L],
                        in1=pref[:, :, 0:SL:L],
                        op=Alu.subtract)

                # packed sums tile: [P=q, g, jp, e] (fp32)
                sE = spsum.tile([P, N_OFF, NJP, 2], dt.float32, tag="sE")
                for gi in range(N_OFF):
                    u = upool.tile([P, NJP * P], dt.float32, tag="u")
                    nc.scalar.activation(u[:], xT[:], Act.Square,
                                         bias=t_bias[gi][:],
                                         scale=t_sqb[:])
                    z = zpool.tile([P, NJP * P], dt.bfloat16, tag="z")
                    fin = nc.vector if gi < N_F_VE else nc.gpsimd
                    fin.tensor_scalar(out=z[:], in0=u[:],
                                      scalar1=float(H_C), scalar2=0.0,
                                      op0=Alu.subtract, op1=Alu.min)
                    for jp in range(NJP):
                        nc.tensor.matmul(
                            out=sE[:, gi, jp, :],
                            lhsT=z[:, jp * P:(jp + 1) * P],
                            rhs=onesm[:],
                            start=True, stop=True)
                return AbV, sE

              def stage_b(c, AbV, sE):
                # ---------- merged digit decode ----------
                # merged layout: [P, r(RC), g(NLIMB)]
                c0m = dpool.tile([P, RC, NLIMB], dt.float32, tag="c0")
                c1m = dpool.tile([P, RC, NLIMB], dt.float32, tag="c1")
                ddm = dpool.tile([P, RC, NLIMB], dt.float32, tag="dd")

                def stage1(src_flat, src_shaped, reshape, dst_c0, dst_c1,
                           dst_dd, n, tag, tt_eng):
                    ri = dpool.tile([P, n], dt.float32, tag=tag + "ri")
                    c1l = dpool.tile([P, n], dt.float32, tag=tag + "c1")
                    nc.vector._custom_dve(RINT, out=ri[:], in0=src_flat,
                                          s0=1.0, s1=0.25, imm2=MAGIC)
                    tt_eng.tensor_tensor(out=dst_dd, in0=src_shaped,
                                         in1=reshape(ri[:]),
                                         op=Alu.subtract)
                    nc.vector._custom_dve(RINT, out=c1l[:], in0=ri[:],
                                          s0=0.0078125, s1=0.25, imm2=MAGIC)
                    nc.vector.scalar_tensor_tensor(
                        out=dst_c0, in0=reshape(c1l[:]), scalar=-128.0,
                        in1=reshape(ri[:]), op0=Alu.mult, op1=Alu.add)
                    nc.vector.tensor_copy(dst_c1, reshape(c1l[:]))

                # VE side: AbV [P, (s r), g] is r-major ((s r) = j)
                stage1(AbV[:].rearrange("p s r g -> p (s r g)"),
                       AbV[:].rearrange("p s r g -> p (s r) g"),
                       lambda a: a.rearrange("p (r g) -> p r g", g=N_V),
                       c0m[:, :, 0:N_V], c1m[:, :, 0:N_V], ddm[:, :, 0:N_V],
                       RC * N_V, "v", nc.vector)

                # psum side: sE [P, g, jp, e]; dst row r = 2*jp+e, limb N_V+g
                sEs = dpool.tile([P, N_OFF, NJP, 2], dt.float32, tag="sEs")
                nc.scalar.copy(sEs[:], sE[:])
                resh = lambda a: a.rearrange("p (g jp e) -> p g jp e",
                                             jp=NJP, e=2)
                stage1(sEs[:].rearrange("p g jp e -> p (g jp e)"),
                       sEs[:],
                       resh,
                       c0m[:, :, N_V:].rearrange("p (jp e) g -> p g jp e",
                                                 jp=NJP),
                       c1m[:, :, N_V:].rearrange("p (jp e) g -> p g jp e",
                                                 jp=NJP),
                       ddm[:, :, N_V:].rearrange("p (jp e) g -> p g jp e",
                                                 jp=NJP),
                       RC * N_OFF, "o", eng[STAGE1_PSUM_ENGINE])

                # u_i = log(c_i/S' + eps); dd lane folds 128 into scale/combine
                u0 = dpool.tile([P, NA], dt.float32, tag="u0")
                u1 = dpool.tile([P, NA], dt.float32, tag="u1")
                u2 = dpool.tile([P, NA], dt.float32, tag="u2")
                c0f = c0m[:].rearrange("p r g -> p (r g)")
                c1f = c1m[:].rearrange("p r g -> p (r g)")
                ddf = ddm[:].rearrange("p r g -> p (r g)")
                nc.scalar.activation(u0[:], c0f, Act.Ln,
                                     bias=t_eps[:], scale=t_inv[:])
                nc.scalar.activation(u1[:], c1f, Act.Ln,
                                     bias=t_eps[:], scale=t_inv[:])
                nc.scalar.activation(u2[:], ddf, Act.Ln,
                                     bias=t_eps[:], scale=t_inv128[:])

                d0 = dpool.tile([P, 1 + NA], dt.float32, tag="d0")
                d1 = dpool.tile([P, 1 + NA], dt.float32, tag="d1")
                d2 = dpool.tile([P, 1 + NA], dt.float32, tag="d2")
                for (dx, cf, uf) in ((d0, c0f, u0), (d1, c1f, u1),
                                     (d2, ddf, u2)):
                    nc.gpsimd.memset(dx[:, 0:1], 0.0)
                    nc.vector._custom_dve(DOT, out=dx[:, 1:], in0=cf,
                                          in1=uf[:])

                # per-row sums from scan ends: S[r] = d[r,13] - d[r-1,13]
                e0 = epool.tile([P, RC], dt.float32, tag="e0")
                e1 = epool.tile([P, RC], dt.float32, tag="e1")
                e2 = epool.tile([P, RC], dt.float32, tag="e2")
                for (dx, ex) in ((d0, e0), (d1, e1), (d2, e2)):
                    nc.vector.tensor_tensor(
                        out=ex[:], in0=dx[:, NLIMB::NLIMB],
                        in1=dx[:, 0:NA:NLIMB], op=Alu.subtract)

                # entropy = -(E0 + E1 + 128*E2)/S'
                acc = epool.tile([P, RC], dt.float32, tag="acc")
                nc.vector.scalar_tensor_tensor(
                    out=acc[:], in0=e2[:], scalar=128.0, in1=e1[:],
                    op0=Alu.mult, op1=Alu.add)
                nc.vector.tensor_tensor(out=acc[:], in0=acc[:], in1=e0[:],
                                        op=Alu.add)
                eout = epool.tile([P, RC], dt.float32, tag="eout")
                nc.vector.tensor_scalar_mul(eout[:], acc[:],
                                            float(-1.0 / S_PRIME))
                nc.sync.dma_start(out=yv[:, c, :], in_=eout[:])

              for c in range(NCHUNK):
                ab, se = stage_a(c)
                stage_b(c, ab, se)

    nc.finalize()
    return nc


def _build_runner(repeat=1):
    """Cached jitted 8-core runner (modeled on bass2jax.run_bass_via_pjrt,
    but reusing one jitted executable across calls)."""
    import jax
    from jax.sharding import Mesh, PartitionSpec
    from jax.experimental.shard_map import shard_map
    import concourse.bass2jax as b2j

    nc = _build_nc(repeat=repeat)
    b2j.install_neuronx_cc_hook()

    import concourse.mybir as mybir
    partition_name = (nc.partition_id_tensor.name
                      if nc.partition_id_tensor else None)
    in_names, out_names, out_avals, zero_outs = [], [], [], []
    for alloc in nc.m.functions[0].allocations:
        if not isinstance(alloc, mybir.MemoryLocationSet):
            continue
        name = alloc.memorylocations[0].name
        if alloc.kind == "ExternalInput":
            if name != partition_name:
                in_names.append(name)
        elif alloc.kind == "ExternalOutput":
            shape = tuple(alloc.tensor_shape)
            dtype = mybir.dt.np(alloc.dtype)
            out_names.append(name)
            out_avals.append(jax.core.ShapedArray(shape, dtype))
            zero_outs.append(np.zeros(shape, dtype))
    n_params = len(in_names)
    n_outs = len(out_avals)
    all_in_names = in_names + out_names
    if partition_name is not None:
        all_in_names = all_in_names + [partition_name]

    def _body(*args):
        operands = list(args)
        if partition_name is not None:
            operands.append(b2j.partition_id_tensor())
        outs = b2j._bass_exec_p.bind(
            *operands,
            out_avals=tuple(out_avals),
            in_names=tuple(all_in_names),
            out_names=tuple(out_names),
            lowering_input_output_aliases=(),
            sim_require_finite=True,
            sim_require_nnan=True,
            nc=nc,
        )
        return tuple(outs)

    devices = jax.devices()[:NCORES]
    mesh = Mesh(np.asarray(devices), ("core",))
    sharded = jax.jit(
        shard_map(_body, mesh=mesh,
                  in_specs=(PartitionSpec("core"),) * (n_params + n_outs),
                  out_specs=(PartitionSpec("core"),) * n_outs,
                  check_rep=False),
        donate_argnums=tuple(range(n_params, n_params + n_outs)),
        keep_unused=True,
    )

    def run(x_full: np.ndarray) -> np.ndarray:
        zeros = [np.zeros((NCORES * z.shape[0], *z.shape[1:]), z.dtype)
                 for z in zero_outs]
        out = sharded(x_full, *zeros)
        return np.asarray(out[0])

    run.sharded = sharded
    run.zero_outs = zero_outs
    run.mesh = mesh
    return run


def kernel(x: np.ndarray) -> np.ndarray:
    global _RUNNER
    x = np.asarray(x)
    assert x.shape == (B, L), x.shape
    if x.dtype != np.int32:
        x = x.astype(np.int32)
    if _RUNNER is None:
        _RUNNER = _build_runner()
    try:
        out = _RUNNER(x)
    except Exception:
        # transient device hiccups (NRT exec-unit resets) have been observed
        # once on this fabric; one retry after a short pause recovers.
        import time
        time.sleep(20.0)
        out = _RUNNER(x)
    return out.reshape(B, 1).astype(np.float32)


if __name__ == "__main__":
    rng = np.random.default_rng(0)
    xa = rng.integers(0, VOCAB, size=(B, L)).astype(np.int32)
    out = kernel(x=xa)
    cnt = np.zeros((B, VOCAB), np.float64)
    for v in range(VOCAB):
        cnt[:, v] = (xa == v).sum(1)
    p = cnt / S_PRIME
    ref = -(p * np.log(p + EPS)).sum(1, keepdims=True)
    err = np.abs(out - ref).max()
    rel = err / np.abs(ref).max()
    print("selfcheck max abs err:", err, "rel:", rel)


# revision 26
# speedup vs baseline: 1.0650x; 1.0622x over previous
"""Trainium2 Bass kernel for nn_EntropyCalculator (per-row histogram entropy).

x: [262144, 64] int32, values in [0, 40). Output: [262144, 1] float32 per-row
entropy of the value histogram: -sum_v p_v*log(p_v + 1e-8), p = c/(64+1e-8).

Multi-engine strategy (per core, pure data parallel over 8 cores):
  The 40-bin histogram is computed as 14 "limb" channels; limb g packs the
  counts of values {3g, 3g+1, 3g+2} into one fp32 number c0 + 128*c1 + c2/128
  (exact: 21 bits < fp32's 24-bit mantissa). The per-element encode is the
  window parabola P(t) = 1 + A*t + B*t^2 (t = x - 3g), which equals the digit
  weight at t = 0,1,2 and is negative at every other integer in range.

  Channels are split across engines:
   * N_V channels run on VectorE as one fused custom DVE op per channel
     (relu-parabola + prefix scan; per-row sums via boundary differences).
   * N_OFF channels run on the transposed path: TensorE transposes x-blocks
     into PSUM; ScalarE computes u = Square(sqrt|B|*(x - mu_g)) (the free
     affine input of ACT); a one-op tensor_scalar finisher (GP or VE) forms
     z = min(u - h, 0) in {0, -1, -128, -1/128} (bf16-exact); TensorE then
     row-sums z by loading it as the stationary operand and multiplying a
     [-1] moving vector, landing packed sums in PSUM with rows on partitions
     (so no partition-crossing repack is needed).
  Decode (digit extraction via exact rint, ACT-Ln, fused multiply-scans) is
  shared, operating on the merged [P, rows, 14] layout.
"""

import numpy as np

VOCAB = 40
L = 64
B = 262144
NCORES = 8
ROWS_PC = B // NCORES          # 32768 rows per core
P = 128                        # SBUF partitions
RPP = ROWS_PC // P             # 256 rows per partition
RC = 32                        # rows per partition per chunk
NCHUNK = RPP // RC             # 8 chunks
SCANROWS = 16                  # rows per scan instruction (fp32 exactness cap)
NSUB = RC // SCANROWS          # 2 scan sub-chunks per chunk
NLIMB = 14
EPS = 1e-8
S_PRIME = 64.0 + EPS

# --- engine split config (tunable) ---
N_V = 5                        # limbs 0..N_V-1 on VectorE fused scans
N_OFF = NLIMB - N_V            # limbs N_V.. on the transposed SE/TE path
N_F_VE = 14                    # of the offloaded, how many finishers on VE
CONV_ENGINE = "vector"         # int32->bf16 conversion engine: gpsimd|vector|scalar
STAGE1_PSUM_ENGINE = "gpsimd"  # psum-side digit stage helpers engine for tt ops

# parabola through (1, 128, 1/128) at t=0,1,2; negative at all other ints
A_C = 254.49609375
B_C = -127.49609375
ABS_B = -B_C                               # 127.49609375
V_C = A_C / (2.0 * ABS_B)                  # vertex offset ~0.99806
H_C = 1.0 + A_C * A_C / (4.0 * ABS_B)     # peak value ~128.0078
SQRT_B = float(np.sqrt(ABS_B))
MAGIC = 8388608.0              # 2^23: rint via (x + 2^23) - 2^23

_RUNNER = None


def _register_ops():
    import concourse.dve_ops as dve_ops
    from concourse.dve_spec import (
        Spec, Src0, Src1, C0, C1, C2, One, scan, AluOp, lower, _has_src1, sq,
        relu,
    )
    from concourse.dve_uop import DveOpSpec

    def reg(name, spec, subdim=False):
        for op in dve_ops.OPS:
            if op.name == name:
                return op
        row = dve_ops._CUSTOM_DVE_ROW_BASE + len(dve_ops.OPS)
        assert row < 0x20, "out of custom-DVE opcode rows"
        shas = {}
        for ver in ("v3", "v4"):
            s = DveOpSpec(name=name, opcode=row, uops=lower(spec, ver=ver),
                          rd1_en=_has_src1(spec))
            shas[ver] = s.sha(ver)
        op = dve_ops.DveOp(name, spec, subdim=subdim, uops_sha=shas)
        dve_ops.OPS.append(op)
        dve_ops.CUSTOM_DVE_SPECS[name] = spec
        dve_ops._SUB_OPCODE_FOR_NAME[name] = row
        return op

    _t = Src0 - C0

    def _ref_limb(in0, in1, s0, s1, imm2):
        t = in0.astype(np.float64) - s0
        z = np.maximum(1.0 + t * s1 + t * t * imm2, 0.0)
        return np.cumsum(z.reshape(z.shape[0], -1), axis=1).astype(np.float32)

    limb = reg("ENT_LIMB_SCAN", Spec(
        body=scan(AluOp.ADD, relu(One + _t * C1 + sq(_t) * C2)),
        reference=_ref_limb))

    def _ref_rint(in0, in1, s0, s1, imm2):
        y = (in0.astype(np.float32) * np.float32(s0)) - np.float32(s1)
        return ((y + np.float32(imm2)) - np.float32(imm2)).astype(np.float32)

    rint = reg("ENT_RINT_AFFINE", Spec(
        body=(Src0 * C0 - C1 + C2) - C2,
        reference=_ref_rint))

    def _ref_dot(in0, in1, s0, s1, imm2):
        z = in0.astype(np.float64) * in1.astype(np.float64)
        return np.cumsum(z.reshape(z.shape[0], -1), axis=1).astype(np.float32)

    dot = reg("ENT_DOT_SCAN", Spec(
        body=scan(AluOp.ADD, Src0 * Src1),
        reference=_ref_dot))

    return limb, rint, dot


def _build_nc(repeat=1):
    from contextlib import ExitStack, nullcontext
    import concourse.bacc as bacc
    import concourse.mybir as mybir
    from concourse.tile import TileContext
    from concourse import masks

    LIMB, RINT, DOT = _register_ops()
    dt = mybir.dt
    Alu = mybir.AluOpType
    Act = mybir.ActivationFunctionType

    nc = bacc.Bacc()
    x = nc.dram_tensor("x", [ROWS_PC, L], dt.int32, kind="ExternalInput")
    y = nc.dram_tensor("y", [ROWS_PC, 1], dt.float32, kind="ExternalOutput")

    # partition p owns rows [p*RPP, (p+1)*RPP); chunk c covers rows c*RC..+RC
    xv = x[:].rearrange("(p c r) l -> p c (r l)", p=P, c=NCHUNK)   # [P, NCHUNK, RC*L]
    yv = y[:].rearrange("(p c r) o -> p c (r o)", p=P, c=NCHUNK)   # [P, NCHUNK, RC]

    NA = RC * NLIMB            # 448 decode values per partition per chunk
    inv_sp = float(1.0 / S_PRIME)
    NJP = RC // 2              # 16 j-pairs per chunk

    eng = {"gpsimd": nc.gpsimd, "vector": nc.vector, "scalar": nc.scalar}

    with TileContext(nc) as tc:
        with ExitStack() as ctx:
            xpool = ctx.enter_context(tc.tile_pool(name="xp", bufs=3))
            bpool = ctx.enter_context(tc.tile_pool(name="bp", bufs=2))
            ppool = ctx.enter_context(tc.tile_pool(name="pp", bufs=3))
            upool = ctx.enter_context(tc.tile_pool(name="up", bufs=4))
            zpool = ctx.enter_context(tc.tile_pool(name="zp", bufs=4))
            apool = ctx.enter_context(tc.tile_pool(name="ap", bufs=2))
            dpool = ctx.enter_context(tc.tile_pool(name="dp", bufs=2))
            epool = ctx.enter_context(tc.tile_pool(name="ep", bufs=2))
            tpsum = ctx.enter_context(tc.tile_pool(name="tp", bufs=2,
                                                   space="PSUM"))
            spsum = ctx.enter_context(tc.tile_pool(name="sp", bufs=2,
                                                   space="PSUM"))
            singles = ctx.enter_context(tc.tile_pool(name="sg", bufs=1))

            t_eps = singles.tile([P, 1], dt.float32)
            nc.vector.memset(t_eps[:], EPS)
            t_inv = singles.tile([P, 1], dt.float32)
            nc.vector.memset(t_inv[:], inv_sp)
            t_inv128 = singles.tile([P, 1], dt.float32)
            nc.vector.memset(t_inv128[:], float(128.0 / S_PRIME))

            ident = singles.tile([P, P], dt.bfloat16)
            masks.make_identity(nc, ident[:])

            t_sqb = singles.tile([P, 1], dt.float32)
            nc.vector.memset(t_sqb[:], float(SQRT_B))
            t_bias = []
            for gi in range(N_OFF):
                mu = 3.0 * (N_V + gi) + V_C
                tb = singles.tile([P, 1], dt.float32, tag=f"tb{gi}")
                nc.vector.memset(tb[:], float(-SQRT_B * mu))
                t_bias.append(tb)

            # moving operand for row sums: col e has -1 on partitions
            # [64e, 64e+64) and 0 elsewhere
            onesm = singles.tile([P, 2], dt.bfloat16)
            nc.vector.memset(onesm[:], 0.0)
            nc.vector.memset(onesm[0:64, 0:1], -1.0)
            nc.vector.memset(onesm[64:128, 1:2], -1.0)

            repctx = tc.For_i(0, repeat, 1) if repeat > 1 else nullcontext()
            with repctx:
              def stage_a(c):
                xt = xpool.tile([P, RC * L], dt.int32, tag="x")
                nc.sync.dma_start(out=xt[:], in_=xv[:, c, :])

                # conversion first: unblocks TE transposes
                xb = bpool.tile([P, RC * L], dt.bfloat16, tag="xb")
                if CONV_ENGINE == "scalar":
                    nc.scalar.copy(xb[:], xt[:])
                else:
                    eng[CONV_ENGINE].tensor_copy(xb[:], xt[:])

                # x_T[64e+l, jp*128+q] = x[row q*RPP + c*RC + 2jp+e, l]
                xT = tpsum.tile([P, NJP * P], dt.bfloat16, tag="xT")
                for jp in range(NJP):
                    nc.tensor.matmul(
                        out=xT[:, jp * P:(jp + 1) * P],
                        lhsT=xb[:, jp * 2 * L:(jp + 1) * 2 * L],
                        rhs=ident[:],
                        is_transpose=True)

                # VE scan path issued BEFORE the finishers so VectorE has
                # immediate work while ScalarE runs the squares.
                AbV = apool.tile([P, NSUB, SCANROWS, N_V], dt.float32,
                                 tag="AV")
                SL = SCANROWS * L
                for g in range(N_V):
                    pref = ppool.tile([P, NSUB, 1 + SL], dt.float32,
                                      tag="pref")
                    nc.gpsimd.memset(pref[:, :, 0:1], 0.0)
                    for s in range(NSUB):
                        nc.vector._custom_dve(
                            LIMB,
                            out=pref[:, s, 1:],
                            in0=xt[:, s * SL:(s + 1) * SL],
                            s0=float(3 * g), s1=A_C, imm2=B_C)
                    nc.gpsimd.tensor_tensor(
                        out=AbV[:, :, :, g],
                        in0=pref[:, :, L::L],
                        in1=pref[:, :, 0:SL:L],
                        op=Alu.subtract)

                # packed sums tile: [P=q, g, jp, e] (fp32)
                sE = spsum.tile([P, N_OFF, NJP, 2], dt.float32, tag="sE")
                for gi in range(N_OFF):
                    u = upool.tile([P, NJP * P], dt.float32, tag="u")
                    nc.scalar.activation(u[:], xT[:], Act.Square,
                                         bias=t_bias[gi][:],
                                         scale=t_sqb[:])
                    z = zpool.tile([P, NJP * P], dt.bfloat16, tag="z")
                    fin = nc.vector if gi < N_F_VE else nc.gpsimd
                    fin.tensor_scalar(out=z[:], in0=u[:],
                                      scalar1=float(H_C), scalar2=0.0,
                                      op0=Alu.subtract, op1=Alu.min)
                    for jp in range(NJP):
                        nc.tensor.matmul(
                            out=sE[:, gi, jp, :],
                            lhsT=z[:, jp * P:(jp + 1) * P],
                            rhs=onesm[:],
                            start=True, stop=True)
                return AbV, sE

              def stage_b(c, AbV, sE):
                # ---------- merged digit decode ----------
                # merged layout: [P, r(RC), g(NLIMB)]
                c0m = dpool.tile([P, RC, NLIMB], dt.float32, tag="c0")
                c1m = dpool.tile([P, RC, NLIMB], dt.float32, tag="c1")
                ddm = dpool.tile([P, RC, NLIMB], dt.float32, tag="dd")

                def stage1(src_flat, src_shaped, reshape, dst_c0, dst_c1,
                           dst_dd, n, tag, tt_eng):
                    ri = dpool.tile([P, n], dt.float32, tag=tag + "ri")
                    c1l = dpool.tile([P, n], dt.float32, tag=tag + "c1")
                    nc.vector._custom_dve(RINT, out=ri[:], in0=src_flat,
                                          s0=1.0, s1=0.25, imm2=MAGIC)
                    tt_eng.tensor_tensor(out=dst_dd, in0=src_shaped,
                                         in1=reshape(ri[:]),
                                         op=Alu.subtract)
                    nc.vector._custom_dve(RINT, out=c1l[:], in0=ri[:],
                                          s0=0.0078125, s1=0.25, imm2=MAGIC)
                    nc.vector.scalar_tensor_tensor(
                        out=dst_c0, in0=reshape(c1l[:]), scalar=-128.0,
                        in1=reshape(ri[:]), op0=Alu.mult, op1=Alu.add)
                    nc.vector.tensor_copy(dst_c1, reshape(c1l[:]))

                # VE side: AbV [P, (s r), g] is r-major ((s r) = j)
                stage1(AbV[:].rearrange("p s r g -> p (s r g)"),
                       AbV[:].rearrange("p s r g -> p (s r) g"),
                       lambda a: a.rearrange("p (r g) -> p r g", g=N_V),
                       c0m[:, :, 0:N_V], c1m[:, :, 0:N_V], ddm[:, :, 0:N_V],
                       RC * N_V, "v", nc.gpsimd)

                # psum side: sE [P, g, jp, e]; dst row r = 2*jp+e, limb N_V+g
                sEs = dpool.tile([P, N_OFF, NJP, 2], dt.float32, tag="sEs")
                nc.scalar.copy(sEs[:], sE[:])
                resh = lambda a: a.rearrange("p (g jp e) -> p g jp e",
                                             jp=NJP, e=2)
                stage1(sEs[:].rearrange("p g jp e -> p (g jp e)"),
                       sEs[:],
                       resh,
                       c0m[:, :, N_V:].rearrange("p (jp e) g -> p g jp e",
                                                 jp=NJP),
                       c1m[:, :, N_V:].rearrange("p (jp e) g -> p g jp e",
                                                 jp=NJP),
                       ddm[:, :, N_V:].rearrange("p (jp e) g -> p g jp e",
                                                 jp=NJP),
                       RC * N_OFF, "o", eng[STAGE1_PSUM_ENGINE])

                # u_i = log(c_i/S' + eps); dd lane folds 128 into scale/combine
                u0 = dpool.tile([P, NA], dt.float32, tag="u0")
                u1 = dpool.tile([P, NA], dt.float32, tag="u1")
                u2 = dpool.tile([P, NA], dt.float32, tag="u2")
                c0f = c0m[:].rearrange("p r g -> p (r g)")
                c1f = c1m[:].rearrange("p r g -> p (r g)")
                ddf = ddm[:].rearrange("p r g -> p (r g)")
                nc.scalar.activation(u0[:], c0f, Act.Ln,
                                     bias=t_eps[:], scale=t_inv[:])
                nc.scalar.activation(u1[:], c1f, Act.Ln,
                                     bias=t_eps[:], scale=t_inv[:])
                nc.scalar.activation(u2[:], ddf, Act.Ln,
                                     bias=t_eps[:], scale=t_inv128[:])

                d0 = dpool.tile([P, 1 + NA], dt.float32, tag="d0")
                d1 = dpool.tile([P, 1 + NA], dt.float32, tag="d1")
                d2 = dpool.tile([P, 1 + NA], dt.float32, tag="d2")
                for (dx, cf, uf) in ((d0, c0f, u0), (d1, c1f, u1),
                                     (d2, ddf, u2)):
                    nc.gpsimd.memset(dx[:, 0:1], 0.0)
                    nc.vector._custom_dve(DOT, out=dx[:, 1:], in0=cf,
                                          in1=uf[:])

                # per-row sums from scan ends: S[r] = d[r,13] - d[r-1,13]
                e0 = epool.tile([P, RC], dt.float32, tag="e0")
                e1 = epool.tile([P, RC], dt.float32, tag="e1")
                e2 = epool.tile([P, RC], dt.float32, tag="e2")
                for (dx, ex) in ((d0, e0), (d1, e1), (d2, e2)):
                    nc.gpsimd.tensor_tensor(
                        out=ex[:], in0=dx[:, NLIMB::NLIMB],
                        in1=dx[:, 0:NA:NLIMB], op=Alu.subtract)

                # entropy = -(E0 + E1 + 128*E2)/S'
                acc = epool.tile([P, RC], dt.float32, tag="acc")
                nc.vector.scalar_tensor_tensor(
                    out=acc[:], in0=e2[:], scalar=128.0, in1=e1[:],
                    op0=Alu.mult, op1=Alu.add)
                nc.gpsimd.tensor_tensor(out=acc[:], in0=acc[:], in1=e0[:],
                                        op=Alu.add)
                eout = epool.tile([P, RC], dt.float32, tag="eout")
                nc.vector.tensor_scalar_mul(eout[:], acc[:],
                                            float(-1.0 / S_PRIME))
                nc.sync.dma_start(out=yv[:, c, :], in_=eout[:])

              for c in range(NCHUNK):
                ab, se = stage_a(c)
                stage_b(c, ab, se)

    nc.finalize()
    return nc


def _build_runner(repeat=1):
    """Cached jitted 8-core runner (modeled on bass2jax.run_bass_via_pjrt,
    but reusing one jitted executable across calls)."""
    import jax
    from jax.sharding import Mesh, PartitionSpec
    from jax.experimental.shard_map import shard_map
    import concourse.bass2jax as b2j

    nc = _build_nc(repeat=repeat)
    b2j.install_neuronx_cc_hook()

    import concourse.mybir as mybir
    partition_name = (nc.partition_id_tensor.name
                      if nc.partition_id_tensor else None)
    in_names, out_names, out_avals, zero_outs = [], [], [], []
    for alloc in nc.m.functions[0].allocations:
        if not isinstance(alloc, mybir.MemoryLocationSet):
            continue
        name = alloc.memorylocations[0].name
        if alloc.kind == "ExternalInput":
            if name != partition_name:
                in_names.append(name)
        elif alloc.kind == "ExternalOutput":
            shape = tuple(alloc.tensor_shape)
            dtype = mybir.dt.np(alloc.dtype)
            out_names.append(name)
            out_avals.append(jax.core.ShapedArray(shape, dtype))
            zero_outs.append(np.zeros(shape, dtype))
    n_params = len(in_names)
    n_outs = len(out_avals)
    all_in_names = in_names + out_names
    if partition_name is not None:
        all_in_names = all_in_names + [partition_name]

    def _body(*args):
        operands = list(args)
        if partition_name is not None:
            operands.append(b2j.partition_id_tensor())
        outs = b2j._bass_exec_p.bind(
            *operands,
            out_avals=tuple(out_avals),
            in_names=tuple(all_in_names),
            out_names=tuple(out_names),
            lowering_input_output_aliases=(),
            sim_require_finite=True,
            sim_require_nnan=True,
            nc=nc,
        )
        return tuple(outs)

    devices = jax.devices()[:NCORES]
    mesh = Mesh(np.asarray(devices), ("core",))
    sharded = jax.jit(
        shard_map(_body, mesh=mesh,
                  in_specs=(PartitionSpec("core"),) * (n_params + n_outs),
                  out_specs=(PartitionSpec("core"),) * n_outs,
                  check_rep=False),
        donate_argnums=tuple(range(n_params, n_params + n_outs)),
        keep_unused=True,
    )

    def run(x_full: np.ndarray) -> np.ndarray:
        zeros = [np.zeros((NCORES * z.shape[0], *z.shape[1:]), z.dtype)
                 for z in zero_outs]
        out = sharded(x_full, *zeros)
        return np.asarray(out[0])

    run.sharded = sharded
    run.zero_outs = zero_outs
    run.mesh = mesh
    return run


def kernel(x: np.ndarray) -> np.ndarray:
    global _RUNNER
    x = np.asarray(x)
    assert x.shape == (B, L), x.shape
    if x.dtype != np.int32:
        x = x.astype(np.int32)
    if _RUNNER is None:
        _RUNNER = _build_runner()
    try:
        out = _RUNNER(x)
    except Exception:
        # transient device hiccups (NRT exec-unit resets) have been observed
        # once on this fabric; one retry after a short pause recovers.
        import time
        time.sleep(20.0)
        out = _RUNNER(x)
    return out.reshape(B, 1).astype(np.float32)


if __name__ == "__main__":
    rng = np.random.default_rng(0)
    xa = rng.integers(0, VOCAB, size=(B, L)).astype(np.int32)
    out = kernel(x=xa)
    cnt = np.zeros((B, VOCAB), np.float64)
    for v in range(VOCAB):
        cnt[:, v] = (xa == v).sum(1)
    p = cnt / S_PRIME
    ref = -(p * np.log(p + EPS)).sum(1, keepdims=True)
    err = np.abs(out - ref).max()
    rel = err / np.abs(ref).max()
    print("selfcheck max abs err:", err, "rel:", rel)
